# revision 1
# baseline (speedup 1.0000x reference)
"""Trainium2 Bass kernel for nn_MultiHeadAttention (B=4, S=2048, H=16, D=64).

Sharding: 8 cores = 4 batches x 2 head-groups (8 heads each). Attention is
fully local per core; the output projection is column-sharded after one
pairwise AllGather of per-head attention outputs between a batch's two cores.

Math folds (all exact):
- Q projection folded away: energy^T = kT^T (wk^T wq) qT /32, so only K is
  projected (with m32 = wk^T wq / 32).
- q/k biases: softmax(e + u[k] + w[q] + c) == softmax(e + u[k]) per column;
  u = kT^T (wk^T bq)/32 rides as row 64 of the projected K (augmented weight
  column), paired with a ones row baked into qT.
- V projection folded PAST attention: O = wv (Vnat^T P); a ones column in
  natural-layout V makes row 64 of R = Vnat^T P the softmax denominator.
- v bias: folds into bo_eff = bo + wo @ tile(bv) host-side.

fp32r everywhere (11-bit mantissa, full PE rate at N>=256, even dst free).
"""

import numpy as np

import concourse.bass as bass
import concourse.mybir as mybir
import concourse.tile as tile
from concourse import bacc
from concourse.bass_utils import run_bass_kernel_spmd

f32 = mybir.dt.float32
f32r = mybir.dt.float32r

B, S, H, D = 4, 2048, 16, 64
HPC = 8  # heads per core
NB = 512  # matmul moving-dim chunk (psum bank limit for fp32 out)
NKB = S // 128  # 16 k-blocks
EXP = mybir.ActivationFunctionType.Exp


def round_fp32r(x: np.ndarray) -> np.ndarray:
    b = np.ascontiguousarray(x.astype(np.float32)).view(np.uint32)
    return ((b + 0x800) & 0xFFFFF000).view(np.float32)


def build(reps=1, use_cc=True, qb_size=1024):
    nc = bacc.Bacc("TRN2", target_bir_lowering=False, num_devices=8)

    kt = nc.dram_tensor("kt", [HPC, D, S], f32r, kind="ExternalInput")
    qt = nc.dram_tensor("qt", [HPC, D + 1, S], f32r, kind="ExternalInput")
    vna = nc.dram_tensor("vna", [HPC, 128, NKB * (D + 1)], f32r, kind="ExternalInput")
    m32u = nc.dram_tensor("m32u", [D, D + 1], f32r, kind="ExternalInput")
    wvt = nc.dram_tensor("wvt", [D, D], f32r, kind="ExternalInput")
    wot = nc.dram_tensor("wot", [HPC, 128, 512], f32r, kind="ExternalInput")
    boe = nc.dram_tensor("boe", [128, 4], f32, kind="ExternalInput")
    out = nc.dram_tensor("out", [512, S], f32, kind="ExternalOutput")

    with tile.TileContext(nc) as tc:
        with tc.tile_pool(name="dram", bufs=1, space="DRAM") as dram:
            for r in range(reps):
                ccin = dram.tile([HPC * D, S], f32r, tag=f"ccin{r}", name=f"ccin{r}")
                ccout = dram.tile(
                    [2 * HPC * D, S], f32r, tag=f"ccout{r}", name=f"ccout{r}"
                )
                _phase_ab(nc, tc, kt, qt, vna, m32u, wvt, ccin, ccout, use_cc, qb_size)
                _phase_c(nc, tc, wot, boe, out, ccout)
    nc.compile()
    return nc


def _phase_ab(nc, tc, kt, qt, vna, m32u, wvt, ccin, ccout, use_cc, QB):
    nqb = S // QB
    sc_bufs = 2 if QB <= 1024 else 1
    rp_bufs = 2 if QB <= 1024 else 1
    with tc.tile_pool(name="keep", bufs=1) as keep:
        m32u_s = keep.tile([D, D + 1], f32r, tag="m32u")
        nc.default_dma_engine.dma_start(out=m32u_s, in_=m32u[:])
        wvt_s = keep.tile([D, D], f32r, tag="wvt")
        nc.default_dma_engine.dma_start(out=wvt_s, in_=wvt[:])

        Qraw = [
            keep.tile([D + 1, S], f32r, tag=f"Qraw{p}", name=f"Qraw{p}")
            for p in range(HPC)
        ]
        Ks = [
            keep.tile([D + 1, S], f32r, tag=f"Ks{p}", name=f"Ks{p}")
            for p in range(HPC)
        ]
        Vn = [
            keep.tile([128, NKB, D + 1], f32r, tag=f"Vn{p}", name=f"Vn{p}")
            for p in range(HPC)
        ]

        # ---- Phase A: load raws + project K (m32u -> [K~; u]) ----
        with (
            tc.tile_pool(name="raw", bufs=2) as raw,
            tc.tile_pool(name="psk", bufs=1, space="PSUM") as pskp,
        ):
            for p in range(HPC):
                kt_t = raw.tile([D, S], f32r, tag="kt")
                nc.default_dma_engine.dma_start(out=kt_t, in_=kt[p])
                nc.default_dma_engine.dma_start(out=Qraw[p], in_=qt[p])
                nc.default_dma_engine.dma_start(
                    out=Vn[p], in_=vna[p].rearrange("p (n d) -> p n d", n=NKB)
                )
                psk = pskp.tile([D + 1, S], f32, tag="psk")
                for ch in range(S // NB):
                    nc.tensor.matmul(
                        psk[:, ch * NB : (ch + 1) * NB],
                        lhsT=m32u_s[:],
                        rhs=kt_t[:, ch * NB : (ch + 1) * NB],
                        start=True,
                        stop=True,
                    )
                nc.vector.tensor_copy(Ks[p][:], psk[:])

        # ---- Phase B: attention per pair ----
        with (
            tc.tile_pool(name="bdram", bufs=2, space="DRAM") as bdram,
            tc.tile_pool(name="scp", bufs=sc_bufs, space="PSUM") as scp,
            tc.tile_pool(name="rp", bufs=rp_bufs, space="PSUM") as rp,
            tc.tile_pool(name="pex", bufs=3) as pex,
            tc.tile_pool(name="onp", bufs=2) as onp,
            tc.tile_pool(name="nrm", bufs=1) as nrm,
        ):
            for p in range(HPC):
                On = onp.tile([D, S], f32r, tag="On")
                for qb in range(nqb):
                    q0 = qb * QB
                    R = rp.tile([D + 1, QB], f32, tag="R")
                    for kb in range(NKB):
                        sc = scp.tile([128, QB], f32, tag="sc")
                        for h in range(QB // NB):
                            nc.tensor.matmul(
                                sc[:, h * NB : (h + 1) * NB],
                                lhsT=Ks[p][:, kb * 128 : (kb + 1) * 128],
                                rhs=Qraw[p][:, q0 + h * NB : q0 + (h + 1) * NB],
                                start=True,
                                stop=True,
                            )
                        pt = pex.tile([128, QB], f32r, tag="pt")
                        nc.scalar.activation(pt[:], sc[:], EXP, scale=1.0)
                        for h in range(QB // NB):
                            nc.tensor.matmul(
                                R[:, h * NB : (h + 1) * NB],
                                lhsT=Vn[p][:, kb, :],
                                rhs=pt[:, h * NB : (h + 1) * NB],
                                start=(kb == 0),
                                stop=(kb == NKB - 1),
                            )
                    rcp = nrm.tile([1, QB], f32r, tag="rcp")
                    with nc.allow_low_precision(reason="fp32r softmax denom"):
                        nc.vector.reciprocal(rcp[:], R[D : D + 1, :])
                    bd = bdram.tile([1, QB], f32r, tag="bd")
                    nc.default_dma_engine.dma_start(out=bd[:], in_=rcp[:])
                    bcs = nrm.tile([D, QB], f32r, tag="bcs")
                    bd_bcast = bass.AP(
                        tensor=bd.tensor, offset=bd.offset, ap=[[0, D]] + list(bd.ap)[1:]
                    )
                    nc.default_dma_engine.dma_start(out=bcs[:], in_=bd_bcast)
                    rsb = nrm.tile([D, QB], f32r, tag="rsb")
                    nc.vector.tensor_copy(rsb[:], R[0:D, :])
                    ops = scp.tile([128, QB], f32, tag="sc")
                    for h in range(QB // NB):
                        nc.tensor.matmul(
                            ops[0:D, h * NB : (h + 1) * NB],
                            lhsT=wvt_s[:],
                            rhs=rsb[:, h * NB : (h + 1) * NB],
                            start=True,
                            stop=True,
                        )
                    nc.vector.tensor_mul(
                        On[:, q0 : q0 + QB], ops[0:D, :], bcs[:]
                    )
                nc.default_dma_engine.dma_start(
                    out=ccin[p * D : (p + 1) * D, :], in_=On[:]
                )
            if use_cc:
                nc.gpsimd.collective_compute(
                    "AllGather",
                    mybir.AluOpType.bypass,
                    replica_groups=[[0, 1], [2, 3], [4, 5], [6, 7]],
                    ins=[ccin.opt()],
                    outs=[ccout.opt()],
                )


def _phase_c(nc, tc, wot, boe, out, ccout):
    with (
        tc.tile_pool(name="pcw", bufs=1) as pcw,
        tc.tile_pool(name="pco", bufs=1) as pco,
        tc.tile_pool(name="fin", bufs=2) as finp,
        tc.tile_pool(name="fps", bufs=2, space="PSUM") as fpsp,
    ):
        boe_s = pcw.tile([128, 4], f32, tag="boe")
        nc.default_dma_engine.dma_start(out=boe_s, in_=boe[:])
        wo_s = []
        Ob = []
        for t in range(HPC):
            w = pcw.tile([128, 512], f32r, tag=f"wo{t}", name=f"wo{t}")
            nc.default_dma_engine.dma_start(out=w, in_=wot[t])
            wo_s.append(w)
            o = pco.tile([128, S], f32r, tag=f"Ob{t}", name=f"Ob{t}")
            nc.default_dma_engine.dma_start(
                out=o, in_=ccout[t * 128 : (t + 1) * 128, :]
            )
            Ob.append(o)
        for ob in range(4):
            for qc in range(S // NB):
                fp_ = fpsp.tile([128, NB], f32, tag="fp")
                for t in range(HPC):
                    nc.tensor.matmul(
                        fp_[:],
                        lhsT=wo_s[t][:, ob * 128 : (ob + 1) * 128],
                        rhs=Ob[t][:, qc * NB : (qc + 1) * NB],
                        start=(t == 0),
                        stop=(t == HPC - 1),
                    )
                fo = finp.tile([128, NB], f32, tag="fo")
                nc.vector.tensor_scalar_add(fo[:], fp_[:], boe_s[:, ob : ob + 1])
                nc.default_dma_engine.dma_start(
                    out=out[ob * 128 : (ob + 1) * 128, qc * NB : (qc + 1) * NB],
                    in_=fo[:],
                )


_NC_CACHE = {}


def _get_nc(reps=1, use_cc=True, qb_size=1024):
    key = (reps, use_cc, qb_size)
    if key not in _NC_CACHE:
        _NC_CACHE[key] = build(reps, use_cc, qb_size)
    return _NC_CACHE[key]


def _prep_core_inputs(values, keys, query, wq, bq, wk, bk, wv, bv, wo, bo):
    """Build the 8 per-core input maps (host-side shard + layout prep)."""
    del bk  # cancels in softmax (q-only term)
    m32 = (wk.T.astype(np.float64) @ wq.astype(np.float64)) / 32.0
    wub1 = (wk.T.astype(np.float64) @ bq.astype(np.float64)) / 32.0
    m32u = round_fp32r(np.concatenate([m32, wub1.reshape(D, 1)], axis=1))
    wvt = round_fp32r(wv.T)

    bv_full = np.tile(bv, H)
    bo_eff = (
        bo.astype(np.float64) + wo.astype(np.float64) @ bv_full.astype(np.float64)
    ).astype(np.float32)
    woT = wo.T  # [in 1024, out 1024]

    in_maps = []
    for c in range(8):
        b, g = c // 2, c % 2
        heads = slice(g * HPC, (g + 1) * HPC)
        kt = round_fp32r(keys[b].reshape(S, H, D)[:, heads, :].transpose(1, 2, 0))
        qt = round_fp32r(query[b].reshape(S, H, D)[:, heads, :].transpose(1, 2, 0))
        qta = np.concatenate([qt, np.ones((HPC, 1, S), np.float32)], axis=1)
        vn = round_fp32r(values[b].reshape(S, H, D)[:, heads, :].transpose(1, 0, 2))
        vna = np.concatenate([vn, np.ones((HPC, S, 1), np.float32)], axis=2)
        # [h, S, 65] -> [h, 128 partitions, NKB*65] matching SBUF tile layout
        vna = vna.reshape(HPC, NKB, 128, D + 1).transpose(0, 2, 1, 3).reshape(
            HPC, 128, NKB * (D + 1)
        )
        ocols = slice(g * 512, (g + 1) * 512)
        wot = np.ascontiguousarray(woT[:, ocols].reshape(HPC, 128, 512))
        boe = np.ascontiguousarray(bo_eff[g * 512 : (g + 1) * 512].reshape(4, 128).T)
        in_maps.append(
            dict(
                kt=np.ascontiguousarray(kt),
                qt=np.ascontiguousarray(qta),
                vna=np.ascontiguousarray(vna),
                m32u=m32u,
                wvt=wvt,
                wot=round_fp32r(wot),
                boe=boe,
            )
        )
    return in_maps


def kernel(values, keys, query, wq, bq, wk, bk, wv, bv, wo, bo):
    values = np.asarray(values, np.float32)
    keys = np.asarray(keys, np.float32)
    query = np.asarray(query, np.float32)
    in_maps = _prep_core_inputs(
        values, keys, query,
        np.asarray(wq, np.float32), np.asarray(bq, np.float32),
        np.asarray(wk, np.float32), np.asarray(bk, np.float32),
        np.asarray(wv, np.float32), np.asarray(bv, np.float32),
        np.asarray(wo, np.float32), np.asarray(bo, np.float32),
    )
    nc = _get_nc()
    res = run_bass_kernel_spmd(nc, in_maps, list(range(8)))
    out = np.empty((B, S, 1024), np.float32)
    for c in range(8):
        b, g = c // 2, c % 2
        out[b, :, g * 512 : (g + 1) * 512] = res.results[c]["out"].T
    return out



# revision 2
# speedup vs baseline: 2.3484x; 2.3484x over previous
"""Trainium2 Bass kernel v2 for nn_MultiHeadAttention (B=4, S=2048, H=16, D=64).

Sharding: 8 cores = 4 batches x 2 query-halves (seq-parallel). Each core owns
1024 query rows of one batch, all 16 heads, and produces the FULL 1024 output
columns for its rows — zero cross-core communication (no collective).

Math folds (all exact):
- Both projections folded to the Q side: energy^T = Kraw^T (M Qnat + w 1^T)
  with M = wk^T wq/32, w = wk^T bq/32 — K needs NO on-device projection.
- bk/bq per-query terms drop (softmax shift-invariance per query column).
- V projection folded past attention INTO wo on host: woe = wo @ blockdiag(wv)
  (weights-only), so raw V rides through attention; a ones column in V makes
  row 64 of R = Vnat^T P the softmax denominator.
- v bias: bo_eff = bo + wo @ tile(bv) host-side.

All engine ops sit at partition base 0 (ISA tile-position constraint); the
hidden matrix assembles into 128-partition head-pair blocks via DMA placement
for odd heads so the output GEMM contracts 128 partitions per step.
"""

import numpy as np

try:
    from ml_dtypes import bfloat16 as np_bf16
except ImportError:  # only needed when USE_BF16
    np_bf16 = None

import concourse.bass as bass
import concourse.mybir as mybir
import concourse.tile as tile
from concourse import bacc
from concourse.bass_utils import run_bass_kernel_spmd

f32 = mybir.dt.float32
f32r = mybir.dt.float32r
bf16 = mybir.dt.bfloat16

USE_BF16 = False  # bf16 matmul inputs measured slower (per-mm convert cost)
ENERGY_STOP = False  # stop flag is sim-only per bass docs; test dropping it
MMDT = bf16 if USE_BF16 else f32r

B, S, H, D = 4, 2048, 16, 64
SQ = 1024  # query rows per core
NKB = S // 128  # 16 k-blocks
VW = D + 1  # V block width incl. ones column
EXP = mybir.ActivationFunctionType.Exp


def round_fp32r(x: np.ndarray) -> np.ndarray:
    b = np.ascontiguousarray(x.astype(np.float32)).view(np.uint32)
    return ((b + 0x800) & 0xFFFFF000).view(np.float32)


def to_mmdt(x: np.ndarray) -> np.ndarray:
    if USE_BF16:
        return np.ascontiguousarray(x).astype(np_bf16)
    return round_fp32r(np.ascontiguousarray(x))


def build(reps=1):
    nc = bacc.Bacc("TRN2", target_bir_lowering=False, num_devices=8)

    m32t = nc.dram_tensor("m32t", [D + 1, D], MMDT, kind="ExternalInput")
    qaug = nc.dram_tensor("qaug", [D + 1, H * SQ], MMDT, kind="ExternalInput")
    ktr = nc.dram_tensor("ktr", [H, D, S], MMDT, kind="ExternalInput")
    vna = nc.dram_tensor("vna", [H, 128, NKB * VW], MMDT, kind="ExternalInput")
    woet = nc.dram_tensor("woet", [128, 8192], MMDT, kind="ExternalInput")
    boe = nc.dram_tensor("boe", [128, 8], f32, kind="ExternalInput")
    out = nc.dram_tensor("out", [128, 8192], MMDT, kind="ExternalOutput")

    with tile.TileContext(nc) as tc:
        for r in range(reps):
            _one_rep(nc, tc, m32t, qaug, ktr, vna, woet, boe, out, r)
    nc.compile()
    return nc


def _one_rep(nc, tc, m32t, qaug, ktr, vna, woet, boe, out, r):
    with tc.tile_pool(name=f"keep{r}", bufs=1) as keep:
        On = keep.tile([128, 8192], MMDT, tag="On")

        with tc.tile_pool(name="qkeep", bufs=1) as qkeep:
            Qp = qkeep.tile([D, H * SQ], MMDT, tag="Qp")

            # ---- Phase Q: project Q (+bias fold) for all heads ----
            with (
                tc.tile_pool(name="qraw", bufs=1) as qraw,
                tc.tile_pool(name="psq", bufs=2, space="PSUM") as psq,
            ):
                m32t_s = qraw.tile([D + 1, D], MMDT, tag="m32t")
                nc.default_dma_engine.dma_start(out=m32t_s, in_=m32t[:])
                qaug_s = qraw.tile([D + 1, H * SQ], MMDT, tag="qaug")
                nc.default_dma_engine.dma_start(out=qaug_s, in_=qaug[:])
                for h in range(H):
                    pq = psq.tile([D, 1024], f32, tag="pq")
                    for qc in range(2):
                        nc.tensor.matmul(
                            pq[:, qc * 512 : qc * 512 + 512],
                            lhsT=m32t_s[:],
                            rhs=qaug_s[:, h * SQ + qc * 512 : h * SQ + qc * 512 + 512],
                            start=True,
                            stop=False,
                            skip_group_check=True,
                        )
                    nc.vector.tensor_copy(Qp[:, h * SQ : (h + 1) * SQ], pq[:])

            # ---- Attention per head ----
            with (
                tc.tile_pool(name="kv", bufs=2) as kv,
                tc.tile_pool(name="scp", bufs=2, space="PSUM") as scp,
                tc.tile_pool(name="rp", bufs=2, space="PSUM") as rp,
                tc.tile_pool(name="pex", bufs=2) as pex,
                tc.tile_pool(name="nrm", bufs=2) as nrm,
                tc.tile_pool(name="bdram", bufs=2, space="DRAM") as bdram,
            ):
                for h in range(H):
                    ktr_t = kv.tile([D, S], MMDT, tag="ktr")
                    nc.default_dma_engine.dma_start(out=ktr_t, in_=ktr[h])
                    vna_t = kv.tile([128, NKB * VW], MMDT, tag="vna")
                    nc.default_dma_engine.dma_start(out=vna_t, in_=vna[h])
                    R = rp.tile([VW, 1024], f32, tag="R")
                    for qc in range(2):
                        for kb2 in range(8):
                            sc = scp.tile([128, 1024], f32, tag="sc")
                            for j in range(2):
                                kb = 2 * kb2 + j
                                nc.tensor.matmul(
                                    sc[:, j * 512 : j * 512 + 512],
                                    lhsT=ktr_t[:, kb * 128 : kb * 128 + 128],
                                    rhs=Qp[
                                        :,
                                        h * SQ + qc * 512 : h * SQ + qc * 512 + 512,
                                    ],
                                    start=True,
                                    stop=ENERGY_STOP,
                                    skip_group_check=not ENERGY_STOP,
                                )
                            pt = pex.tile([128, 1024], MMDT, tag="pt")
                            nc.scalar.activation(pt[:], sc[:], EXP, scale=1.0)
                            for j in range(2):
                                kb = 2 * kb2 + j
                                nc.tensor.matmul(
                                    R[:, qc * 512 : qc * 512 + 512],
                                    lhsT=vna_t[:, kb * VW : kb * VW + VW],
                                    rhs=pt[:, j * 512 : j * 512 + 512],
                                    start=(kb == 0),
                                    stop=False,
                                    skip_group_check=True,
                                )
                    # normalize: R[0:64]/R[64] -> On head-pair slot
                    rcp = nrm.tile([1, 1024], f32r, tag="rcp")
                    with nc.allow_low_precision(reason="fp32r softmax denom"):
                        nc.vector.reciprocal(rcp[:], R[D : D + 1, :])
                    bd = bdram.tile([1, 1024], f32r, tag="bd")
                    nc.default_dma_engine.dma_start(out=bd[:], in_=rcp[:])
                    bcs = nrm.tile([D, 1024], f32r, tag="bcs")
                    bd_b = bass.AP(
                        tensor=bd.tensor,
                        offset=bd.offset,
                        ap=[[0, D]] + list(bd.ap)[1:],
                    )
                    nc.default_dma_engine.dma_start(out=bcs[:], in_=bd_b)
                    hp, par = h // 2, h % 2
                    if par == 0:
                        nc.vector.tensor_mul(
                            On[0:D, hp * 1024 : hp * 1024 + 1024], R[0:D, :], bcs[:]
                        )
                    else:
                        tmp = nrm.tile([D, 1024], MMDT, tag="tmp")
                        nc.vector.tensor_mul(tmp[:], R[0:D, :], bcs[:])
                        nc.default_dma_engine.dma_start(
                            out=On[D : 2 * D, hp * 1024 : hp * 1024 + 1024],
                            in_=tmp[:],
                        )

        # ---- Phase C: out = woe^T-blocks @ On + boe ----
        with (
            tc.tile_pool(name="pcw", bufs=1) as pcw,
            tc.tile_pool(name="fin", bufs=1) as finp,
            tc.tile_pool(name="fps", bufs=2, space="PSUM") as fpsp,
        ):
            woet_s = pcw.tile([128, 8192], MMDT, tag="woet")
            nc.default_dma_engine.dma_start(out=woet_s, in_=woet[:])
            boe_s = pcw.tile([128, 8], f32, tag="boe")
            nc.default_dma_engine.dma_start(out=boe_s, in_=boe[:])
            fo = finp.tile([128, 8192], MMDT, tag="fo")
            for ob in range(8):
                fp_ = fpsp.tile([128, 1024], f32, tag="fp")
                for j in range(8):
                    for qc in range(2):
                        nc.tensor.matmul(
                            fp_[:, qc * 512 : qc * 512 + 512],
                            lhsT=woet_s[
                                :, (j * 8 + ob) * 128 : (j * 8 + ob) * 128 + 128
                            ],
                            rhs=On[:, j * 1024 + qc * 512 : j * 1024 + qc * 512 + 512],
                            start=(j == 0),
                            stop=False,
                            skip_group_check=True,
                        )
                nc.vector.tensor_scalar_add(
                    fo[:, ob * 1024 : ob * 1024 + 1024], fp_[:], boe_s[:, ob : ob + 1]
                )
            nc.default_dma_engine.dma_start(out=out[:], in_=fo[:])


_NC_CACHE = {}


def _get_nc(reps=1, use_cc=True):
    key = reps
    if key not in _NC_CACHE:
        _NC_CACHE[key] = build(reps)
    return _NC_CACHE[key]


def _prep_core_inputs(values, keys, query, wq, bq, wk, bk, wv, bv, wo, bo):
    """Build the 8 per-core input maps (host-side shard + layout prep)."""
    del bk  # drops in softmax (per-query constant)
    M = (wk.T.astype(np.float64) @ wq.astype(np.float64)) / 32.0
    w = (wk.T.astype(np.float64) @ bq.astype(np.float64)) / 32.0
    # lhsT for Q projection: rows = [M^T; w^T], so lhsT^T @ [Qnat;1] = M Qnat + w 1^T
    m32t = to_mmdt(np.concatenate([M.T, w.reshape(1, D)], axis=0))

    bv_full = np.tile(bv, H)
    bo_eff = (
        bo.astype(np.float64) + wo.astype(np.float64) @ bv_full.astype(np.float64)
    ).astype(np.float32)
    # fold wv past attention into wo: woe[:, h-block] = wo[:, h-block] @ wv
    woe = (
        wo.astype(np.float64).reshape(1024, H, D) @ wv.astype(np.float64)
    ).reshape(1024, 1024)
    # lhsT tiles for phase C: woet[p, (j*8+ob)*128 + c] = woe[ob*128+c, j*128+p]
    woeT = woe.T.astype(np.float32)  # [hid, out]
    woet = np.empty((128, 8192), np.float32)
    for j in range(8):
        for ob in range(8):
            woet[:, (j * 8 + ob) * 128 : (j * 8 + ob) * 128 + 128] = woeT[
                j * 128 : j * 128 + 128, ob * 128 : ob * 128 + 128
            ]
    woet = to_mmdt(woet)
    boe_l = np.ascontiguousarray(bo_eff.reshape(8, 128).T)  # [128, 8]

    in_maps = []
    for c in range(8):
        b, half = c // 2, c % 2
        rows = slice(half * SQ, (half + 1) * SQ)
        # qaug [65, 16*1024]: cols h*1024+r; partitions: d (then ones row)
        qt = query[b, rows].reshape(SQ, H, D).transpose(1, 2, 0)  # [H, D, SQ]
        qa = np.concatenate([qt, np.ones((H, 1, SQ), np.float32)], axis=1)
        qaug_c = to_mmdt(qa.transpose(1, 0, 2).reshape(D + 1, H * SQ))
        # ktr [H, D, S]
        ktr_c = to_mmdt(keys[b].reshape(S, H, D).transpose(1, 2, 0))
        # vna [H, 128, NKB*VW]: [h, p, kb*VW + c]
        vn = values[b].reshape(NKB, 128, H, D)  # [kb, p, h, d]
        vna_c = np.concatenate(
            [vn, np.ones((NKB, 128, H, 1), np.float32)], axis=3
        ).transpose(2, 1, 0, 3).reshape(H, 128, NKB * VW)
        in_maps.append(
            dict(
                m32t=m32t,
                qaug=np.ascontiguousarray(qaug_c),
                ktr=np.ascontiguousarray(ktr_c),
                vna=to_mmdt(vna_c),
                woet=woet,
                boe=boe_l,
            )
        )
    return in_maps


def kernel(values, keys, query, wq, bq, wk, bk, wv, bv, wo, bo):
    values = np.asarray(values, np.float32)
    keys = np.asarray(keys, np.float32)
    query = np.asarray(query, np.float32)
    in_maps = _prep_core_inputs(
        values, keys, query,
        np.asarray(wq, np.float32), np.asarray(bq, np.float32),
        np.asarray(wk, np.float32), np.asarray(bk, np.float32),
        np.asarray(wv, np.float32), np.asarray(bv, np.float32),
        np.asarray(wo, np.float32), np.asarray(bo, np.float32),
    )
    nc = _get_nc()
    res = run_bass_kernel_spmd(nc, in_maps, list(range(8)))
    out = np.empty((B, S, 1024), np.float32)
    for c in range(8):
        b, half = c // 2, c % 2
        arr = res.results[c]["out"].astype(np.float32).reshape(128, 8, SQ)
        out[b, half * SQ : (half + 1) * SQ, :] = arr.transpose(2, 1, 0).reshape(
            SQ, 1024
        )
    return out
